# revision 19
# baseline (speedup 1.0000x reference)
"""Trainium2 Bass kernel for nn_Attention_13426067767620 (sparse_attention).

Strategy: data-parallel over batch (B=8 -> one batch element per NeuronCore).
Per core, full attention for 8 heads x 1024 tokens:
  - QKV projection as fp32 PE matmuls (exact), with the module's interleaved
    reshape folded into a host-side row permutation of Wqkv.
  - Relative-position bias via the diamond structure of the bucket map:
    qb = Q @ (bias_table - bias_table[0])^T over the 545 in-diamond offsets
    (plus a bucket-0 column), scattered into per-query score positions with
    gpsimd local_scatter (fp32 values as interleaved int16 index pairs);
    the bucket-0 background is a per-row scalar folded into the ACT
    psum->SBUF copy bias. Scatters run on Pool concurrent with DVE top-k.
  - Exact per-row top-64 via 8 rounds of DVE max8 + match_replace (the
    only engine with match hardware; 15 scans/row-tile is the floor).
  - Softmax over kept entries with tie-exact denominator (row-sum
    accumulated from the masked exp pass itself).
  - P transposed in 128x128 blocks on PE; PV and both projections on PE,
    all matmuls in full fp32 (fp32r is tf32-like and would corrupt the
    top-k boundary).
  Cost-model span ~1.32 ms/core; DVE-bound at 97% utilization.
"""
import os
import sys

sys.path.insert(0, "/opt/trn_rl_repo")
os.environ.setdefault("JAX_PLATFORMS", "")

import numpy as np

NUM_BUCKETS = 33
H = 8
D = 64
DIM = 512
S = 1024
B = 8
GRID = 32
TOPK = 64
SCALE = DIM ** (-0.5)
NQT = S // 128          # 8 query tiles per core
NDIA = 545              # diamond offsets with hamming <= 16
NBT = NDIA + 1          # + bucket-0 column

_cache = {}


def _diamond():
    offs = []
    half = NUM_BUCKETS // 2  # 16
    for rv in range(-half, half + 1):
        w = half - abs(rv)
        for rh in range(-w, w + 1):
            offs.append((rv, rh))
    assert len(offs) == NDIA
    return offs


def _host_prep(hidden_states, Wqkv, Wo, bias_table):
    offs = _diamond()
    half = NUM_BUCKETS // 2

    # weight permutation for the interleaved reshape(B, -1, 3)
    Wq = Wqkv[0::3]   # (512, 512), row e' = h*64+d
    Wk = Wqkv[1::3]
    Wv = Wqkv[2::3]
    wqT = np.ascontiguousarray(Wq.T)   # (dim, e)
    wkT = np.ascontiguousarray(Wk.T)
    wvT = np.ascontiguousarray(Wv.T)
    woT = np.ascontiguousarray(Wo.T)   # (e, dd)

    # bias table, diamond-permuted with the bucket-0 row pre-subtracted
    # (so qb psum is directly the scatter payload), bucket0 itself as the
    # last column; replicated on both 64-partition halves -> (128, 546)
    cols = np.empty((NBT, D), np.float32)
    for j, (rv, rh) in enumerate(offs):
        cols[j] = (bias_table[(rv + half) * NUM_BUCKETS + (rh + half)]
                   - bias_table[0])
    cols[NDIA] = bias_table[0]
    bttT = np.ascontiguousarray(cols.T)          # (64, 546)
    btt2 = np.concatenate([bttT, bttT], axis=0)  # (128, 546)

    # scatter indices: for each q, each diamond col j -> int16 index pair
    # into the int16 view of the fp32 ctx row (split in two 512-fp32 halves)
    q0 = np.arange(S)[:, None] // GRID
    q1 = np.arange(S)[:, None] % GRID
    rv = np.array([o[0] for o in offs])[None, :]
    rh = np.array([o[1] for o in offs])[None, :]
    k0 = q0 + rv
    k1 = q1 + rh
    valid = (k0 >= 0) & (k0 < GRID) & (k1 >= 0) & (k1 < GRID)
    k = k0 * GRID + k1                      # (1024, 545)
    n_half = 2 * NBT                        # 1092 int16 idx slots per half
    sidx = np.full((S, 2, n_half), -1, np.int16)
    for half_i in range(2):
        sel = valid & (k // 512 == half_i)
        kk = (k - half_i * 512) * 2
        jj = np.arange(NDIA) * 2
        for q in range(S):
            m = sel[q]
            sidx[q, half_i, jj[m]] = kk[q, m]
            sidx[q, half_i, jj[m] + 1] = kk[q, m] + 1
    sidx = sidx.reshape(S, 2 * n_half)      # (1024, 2184)

    ident = np.eye(128, dtype=np.float32)
    hsT = np.ascontiguousarray(hidden_states.transpose(0, 2, 1))  # (B, dim, S)
    return hsT, wqT, wkT, wvT, woT, btt2, sidx, ident


def _build():
    from concourse import bacc, mybir, tile

    f32 = mybir.dt.float32
    i16 = mybir.dt.int16
    Alu = mybir.AluOpType
    Act = mybir.ActivationFunctionType

    nc = bacc.Bacc(None, target_bir_lowering=False)
    d_hsT = nc.dram_tensor("hsT", [DIM, S], f32, kind="ExternalInput")
    d_wqT = nc.dram_tensor("wqT", [DIM, DIM], f32, kind="ExternalInput")
    d_wkT = nc.dram_tensor("wkT", [DIM, DIM], f32, kind="ExternalInput")
    d_wvT = nc.dram_tensor("wvT", [DIM, DIM], f32, kind="ExternalInput")
    d_woT = nc.dram_tensor("woT", [DIM, DIM], f32, kind="ExternalInput")
    d_btt = nc.dram_tensor("btt", [128, NBT], f32, kind="ExternalInput")
    d_sidx = nc.dram_tensor("sidx", [S, 4 * NBT], i16, kind="ExternalInput")
    d_id = nc.dram_tensor("ident", [128, 128], f32, kind="ExternalInput")
    d_out = nc.dram_tensor("out", [S, DIM], f32, kind="ExternalOutput")

    with tile.TileContext(nc) as tc:
        with (
            tc.tile_pool(name="const", bufs=1) as cpool,
            tc.tile_pool(name="persist", bufs=1) as ppool,
        ):
            hsT = [cpool.tile([128, S], f32, tag=f"hsT{c}", name=f"hsT{c}") for c in range(4)]
            wq = [cpool.tile([128, DIM], f32, tag=f"wq{c}", name=f"wq{c}") for c in range(4)]
            wk = [cpool.tile([128, DIM], f32, tag=f"wk{c}", name=f"wk{c}") for c in range(4)]
            wv = [cpool.tile([128, DIM], f32, tag=f"wv{c}", name=f"wv{c}") for c in range(4)]
            wo = [cpool.tile([128, DIM], f32, tag=f"wo{c}", name=f"wo{c}") for c in range(4)]
            btt = cpool.tile([128, NBT], f32, tag="btt")
            ident = cpool.tile([128, 128], f32, tag="ident")
            for c in range(4):
                sl = slice(128 * c, 128 * (c + 1))
                nc.sync.dma_start(hsT[c][:], d_hsT[sl, :])
                nc.sync.dma_start(wq[c][:], d_wqT[sl, :])
                nc.sync.dma_start(wk[c][:], d_wkT[sl, :])
                nc.sync.dma_start(wv[c][:], d_wvT[sl, :])
                nc.sync.dma_start(wo[c][:], d_woT[sl, :])
            nc.sync.dma_start(btt[:], d_btt[:])
            nc.sync.dma_start(ident[:], d_id[:])

            # persistent activation layouts
            # QT2/KT2: partition (h%2)*64+d, free (h//2)*1024 + s
            QT2 = ppool.tile([128, 4 * S], f32, tag="QT2")
            KT2 = ppool.tile([128, 4 * S], f32, tag="KT2")
            V = [ppool.tile([128, DIM], f32, tag=f"V{st}", name=f"V{st}") for st in range(8)]
            # o_all tile j covers e in [128j, 128(j+1)), free = q
            o_all = [ppool.tile([128, S], f32, tag=f"oall{j}", name=f"oall{j}") for j in range(4)]

            # ---------------- phase 1: projections ----------------
            with tc.tile_pool(name="ps1", bufs=3, space="PSUM") as ps1:
                for dst, w in ((QT2, wq), (KT2, wk)):
                    for j in range(4):          # head pair
                        for st in range(2):     # s halves of 512
                            ps = ps1.tile([128, 512], f32, tag="proj")
                            for par in range(2):
                                h = 2 * j + par
                                for c in range(4):
                                    nc.tensor.matmul(
                                        ps[64 * par:64 * (par + 1), :],
                                        w[c][:, 64 * h:64 * (h + 1)],
                                        hsT[c][:, 512 * st:512 * (st + 1)],
                                        start=(c == 0), stop=(c == 3),
                                        tile_position=(0, 64 * par),
                                    )
                            nc.scalar.activation(
                                dst[:, j * S + 512 * st: j * S + 512 * (st + 1)],
                                ps[:], Act.Copy)
                for st in range(8):
                    ps = ps1.tile([128, 512], f32, tag="projv")
                    for c in range(4):
                        nc.tensor.matmul(
                            ps[:], hsT[c][:, 128 * st:128 * (st + 1)], wv[c][:],
                            start=(c == 0), stop=(c == 3))
                    nc.scalar.activation(V[st][:], ps[:], Act.Copy)

            # ---------------- phase 2: attention ----------------
            with (
                tc.tile_pool(name="sidxp", bufs=2) as sidxp,
                tc.tile_pool(name="work", bufs=2) as wk2,
                tc.tile_pool(name="pss", bufs=1, space="PSUM") as pss,
                tc.tile_pool(name="psqb", bufs=1, space="PSUM") as psqb,
                tc.tile_pool(name="pst", bufs=2, space="PSUM") as pst,
                tc.tile_pool(name="pso", bufs=2, space="PSUM") as pso,
            ):
                for qt in range(NQT):
                    sidx_t = sidxp.tile([128, 4 * NBT], i16, tag="sidx")
                    nc.sync.dma_start(
                        sidx_t[:], d_sidx[128 * qt:128 * (qt + 1), :])
                    for j in range(4):
                        ps_o = pso.tile([128, 128], f32, tag="pso")
                        for par in range(2):
                            h = 2 * j + par
                            base = 64 * par
                            bsl = slice(base, base + 64)
                            joff = j * S
                            lq = QT2[bsl, joff + 128 * qt: joff + 128 * (qt + 1)]

                            ps_s = pss.tile([128, S], f32, tag="scores")
                            for kb2 in range(2):
                                nc.tensor.matmul(
                                    ps_s[:, 512 * kb2:512 * (kb2 + 1)],
                                    lq,
                                    KT2[bsl, joff + 512 * kb2: joff + 512 * (kb2 + 1)],
                                    start=True, stop=True)
                            ps_qb = psqb.tile([128, 1024], f32, tag="qb")
                            nc.tensor.matmul(ps_qb[:, 0:512], lq,
                                             btt[bsl, 0:512],
                                             start=True, stop=True)
                            nc.tensor.matmul(ps_qb[:, 512:512 + 34], lq,
                                             btt[bsl, 512:NBT],
                                             start=True, stop=True)

                            # psum qb already holds qb_diamond - qb0
                            # (host-side subtraction); copy to SBUF for the
                            # gpsimd scatter, col 545 = qb0.
                            qbd = wk2.tile([128, NBT], f32, tag="qbd")
                            nc.scalar.activation(qbd[:, 0:512],
                                                 ps_qb[:, 0:512], Act.Copy)
                            nc.scalar.activation(qbd[:, 512:NBT],
                                                 ps_qb[:, 512:NBT], Act.Copy)
                            # scatter diamond into ctx (zero elsewhere);
                            # scatters are the ONLY Pool work (HW-measured
                            # ~9.7us each, they run concurrent with DVE topk)
                            ctx = wk2.tile([128, S], f32, tag="ctx")
                            qbd16 = qbd[:].bitcast(i16)
                            ctx16 = ctx[:].bitcast(i16)
                            for hf in range(2):
                                nc.gpsimd.local_scatter(
                                    ctx16[:, 1024 * hf:1024 * (hf + 1)],
                                    qbd16,
                                    sidx_t[:, 2 * NBT * hf:2 * NBT * (hf + 1)],
                                    channels=128, num_elems=1024,
                                    num_idxs=2 * NBT)

                            # qb0 background folded into the psum->SBUF copy
                            # as per-partition ACT bias (same fp32 rounding as
                            # (qk + qb0) + ctx); ctx added on Pool off the
                            # DVE critical path.
                            qb0 = qbd[:, 545:546]
                            sraw_p = wk2.tile([128, S], f32, tag="srawp")
                            nc.scalar.activation(sraw_p[:], ps_s[:],
                                                 Act.Identity, bias=qb0)
                            s_raw = wk2.tile([128, S], f32, tag="sraw")
                            nc.gpsimd.tensor_tensor(
                                s_raw[:], sraw_p[:], ctx[:], op=Alu.add)

                            # exact top-64 per row (last round needs no
                            # match_replace)
                            top64 = wk2.tile([128, 64], f32, tag="top64")
                            scratch = wk2.tile([128, S], f32, tag="scratch")
                            src = s_raw
                            for r in range(8):
                                nc.vector.max(
                                    out=top64[:, 8 * r:8 * (r + 1)],
                                    in_=src[:])
                                if r == 7:
                                    break
                                nc.vector.match_replace(
                                    out=scratch[:],
                                    in_to_replace=top64[:, 8 * r:8 * (r + 1)],
                                    in_values=src[:],
                                    imm_value=-1e30)
                                src = scratch

                            # softmax pieces
                            nsm = wk2.tile([128, 1], f32, tag="nsm")
                            nc.vector.tensor_scalar(
                                nsm[:], top64[:, 0:1], -SCALE, None,
                                op0=Alu.mult)
                            expP = wk2.tile([128, S], f32, tag="expP")
                            nc.scalar.activation(
                                expP[:], s_raw[:], Act.Exp,
                                bias=nsm[:], scale=SCALE)
                            # P = expP * (s_raw >= t64), sigma = row-sum(P)
                            P = wk2.tile([128, S], f32, tag="P")
                            sig = wk2.tile([128, 1], f32, tag="sig")
                            nc.vector.scalar_tensor_tensor(
                                P[:], s_raw[:], top64[:, 63:64], expP[:],
                                op0=Alu.is_ge, op1=Alu.mult,
                                accum_out=sig[:])
                            rs = wk2.tile([128, 1], f32, tag="rs")
                            nc.vector.reciprocal(rs[:], sig[:])
                            nc.gpsimd.tensor_scalar(
                                P[:], P[:], rs[:], None, op0=Alu.mult)

                            # transpose P in 128x128 blocks; PV accumulate
                            for kb in range(8):
                                ps_t = pst.tile([128, 128], f32, tag="pt")
                                nc.tensor.transpose(
                                    ps_t[:], P[:, 128 * kb:128 * (kb + 1)],
                                    ident[:])
                                pt_sb = wk2.tile([128, 128], f32, tag=f"pt{kb % 2}", name=f"ptsb{kb % 2}")
                                nc.scalar.activation(pt_sb[:], ps_t[:], Act.Copy)
                                nc.tensor.matmul(
                                    ps_o[base:base + 64, :],
                                    V[kb][:, 64 * h:64 * (h + 1)],
                                    pt_sb[:],
                                    start=(kb == 0), stop=(kb == 7),
                                    tile_position=(0, base))
                        nc.scalar.activation(
                            o_all[j][:, 128 * qt:128 * (qt + 1)],
                            ps_o[:], Act.Copy)

            # ---------------- phase 3: output projection ----------------
            with (
                tc.tile_pool(name="ps3", bufs=2, space="PSUM") as ps3,
                tc.tile_pool(name="outp", bufs=2) as outp,
            ):
                for st in range(8):
                    ps = ps3.tile([128, 512], f32, tag="out")
                    for c in range(4):
                        nc.tensor.matmul(
                            ps[:], o_all[c][:, 128 * st:128 * (st + 1)],
                            wo[c][:], start=(c == 0), stop=(c == 3))
                    ot = outp.tile([128, 512], f32, tag="ot")
                    nc.scalar.activation(ot[:], ps[:], Act.Copy)
                    nc.sync.dma_start(d_out[128 * st:128 * (st + 1), :], ot[:])

    nc.finalize()
    return nc


def kernel(hidden_states, Wqkv, Wo, bias_table, mask, qs0, qs1, ks0, ks1,
           topk, **_ignored):
    hidden_states = np.asarray(hidden_states, np.float32)
    Wqkv = np.asarray(Wqkv, np.float32)
    Wo = np.asarray(Wo, np.float32)
    bias_table = np.asarray(bias_table, np.float32)

    hsT, wqT, wkT, wvT, woT, btt2, sidx, ident = _host_prep(
        hidden_states, Wqkv, Wo, bias_table)

    if "nc" not in _cache:
        _cache["nc"] = _build()
    nc = _cache["nc"]

    from concourse.bass_utils import run_bass_kernel_spmd
    shared = {"wqT": wqT, "wkT": wkT, "wvT": wvT, "woT": woT,
              "btt": btt2, "sidx": sidx, "ident": ident}
    in_maps = [dict(shared, hsT=np.ascontiguousarray(hsT[b]))
               for b in range(B)]
    res = run_bass_kernel_spmd(nc, in_maps, core_ids=list(range(B)))
    _cache["last_exec_time_ns"] = getattr(res, "exec_time_ns", None)
    out = np.stack([res.results[b]["out"] for b in range(B)], axis=0)
    return out


# revision 20
# speedup vs baseline: 2.6832x; 2.6832x over previous
"""Trainium2 Bass kernel for nn_Attention_13426067767620 (sparse_attention).

Strategy: data-parallel over batch (B=8 -> one batch element per NeuronCore).
Per core, full attention for 8 heads x 1024 tokens:
  - QKV projection as fp32 PE matmuls (exact), with the module's interleaved
    reshape folded into a host-side row permutation of Wqkv.
  - Relative-position bias via the diamond structure of the bucket map:
    qb = Q @ (bias_table - bias_table[0])^T over the 545 in-diamond offsets
    (plus a bucket-0 column), scattered into per-query score positions with
    gpsimd local_scatter (fp32 values as interleaved int16 index pairs);
    the bucket-0 background is a per-row scalar folded into the ACT
    psum->SBUF copy bias. Scatters run on Pool concurrent with DVE top-k.
  - Exact per-row top-64 via 8 rounds of DVE max8 + match_replace (the
    only engine with match hardware; 15 scans/row-tile is the floor).
  - Softmax over kept entries with tie-exact denominator (row-sum
    accumulated from the masked exp pass itself).
  - P transposed in 128x128 blocks on PE; PV and both projections on PE,
    all matmuls in full fp32 (fp32r is tf32-like and would corrupt the
    top-k boundary).
  Cost-model span ~1.32 ms/core; DVE-bound at 97% utilization.
"""
import os
import sys

sys.path.insert(0, "/opt/trn_rl_repo")
os.environ.setdefault("JAX_PLATFORMS", "")

import numpy as np

NUM_BUCKETS = 33
H = 8
D = 64
DIM = 512
S = 1024
B = 8
GRID = 32
TOPK = 64
SCALE = DIM ** (-0.5)
NQT = S // 128          # 8 query tiles per core
NDIA = 545              # diamond offsets with hamming <= 16
NBT = NDIA + 1          # + bucket-0 column

_cache = {}


def _diamond():
    offs = []
    half = NUM_BUCKETS // 2  # 16
    for rv in range(-half, half + 1):
        w = half - abs(rv)
        for rh in range(-w, w + 1):
            offs.append((rv, rh))
    assert len(offs) == NDIA
    return offs


def _host_prep(hidden_states, Wqkv, Wo, bias_table):
    offs = _diamond()
    half = NUM_BUCKETS // 2

    # weight permutation for the interleaved reshape(B, -1, 3)
    Wq = Wqkv[0::3]   # (512, 512), row e' = h*64+d
    Wk = Wqkv[1::3]
    Wv = Wqkv[2::3]
    wqT = np.ascontiguousarray(Wq.T)   # (dim, e)
    wkT = np.ascontiguousarray(Wk.T)
    wvT = np.ascontiguousarray(Wv.T)
    woT = np.ascontiguousarray(Wo.T)   # (e, dd)

    # bias table, diamond-permuted with the bucket-0 row pre-subtracted
    # (so qb psum is directly the scatter payload), bucket0 itself as the
    # last column; replicated on both 64-partition halves -> (128, 546)
    cols = np.empty((NBT, D), np.float32)
    for j, (rv, rh) in enumerate(offs):
        cols[j] = (bias_table[(rv + half) * NUM_BUCKETS + (rh + half)]
                   - bias_table[0])
    cols[NDIA] = bias_table[0]
    bttT = np.ascontiguousarray(cols.T)          # (64, 546)
    btt2 = np.concatenate([bttT, bttT], axis=0)  # (128, 546)

    # scatter indices: for each q, each diamond col j -> int16 index pair
    # into the int16 view of the fp32 ctx row (split in two 512-fp32 halves)
    q0 = np.arange(S)[:, None] // GRID
    q1 = np.arange(S)[:, None] % GRID
    rv = np.array([o[0] for o in offs])[None, :]
    rh = np.array([o[1] for o in offs])[None, :]
    k0 = q0 + rv
    k1 = q1 + rh
    valid = (k0 >= 0) & (k0 < GRID) & (k1 >= 0) & (k1 < GRID)
    k = k0 * GRID + k1                      # (1024, 545)
    n_half = 2 * NBT                        # 1092 int16 idx slots per half
    sidx = np.full((S, 2, n_half), -1, np.int16)
    for half_i in range(2):
        sel = valid & (k // 512 == half_i)
        kk = (k - half_i * 512) * 2
        jj = np.arange(NDIA) * 2
        for q in range(S):
            m = sel[q]
            sidx[q, half_i, jj[m]] = kk[q, m]
            sidx[q, half_i, jj[m] + 1] = kk[q, m] + 1
    sidx = sidx.reshape(S, 2 * n_half)      # (1024, 2184)

    ident = np.eye(128, dtype=np.float32)
    hsT = np.ascontiguousarray(hidden_states.transpose(0, 2, 1))  # (B, dim, S)
    return hsT, wqT, wkT, wvT, woT, btt2, sidx, ident


def _build():
    from concourse import bacc, mybir, tile

    f32 = mybir.dt.float32
    i16 = mybir.dt.int16
    Alu = mybir.AluOpType
    Act = mybir.ActivationFunctionType

    nc = bacc.Bacc(None, target_bir_lowering=False)
    d_hsT = nc.dram_tensor("hsT", [DIM, S], f32, kind="ExternalInput")
    d_wqT = nc.dram_tensor("wqT", [DIM, DIM], f32, kind="ExternalInput")
    d_wkT = nc.dram_tensor("wkT", [DIM, DIM], f32, kind="ExternalInput")
    d_wvT = nc.dram_tensor("wvT", [DIM, DIM], f32, kind="ExternalInput")
    d_woT = nc.dram_tensor("woT", [DIM, DIM], f32, kind="ExternalInput")
    d_btt = nc.dram_tensor("btt", [128, NBT], f32, kind="ExternalInput")
    d_sidx = nc.dram_tensor("sidx", [S, 4 * NBT], i16, kind="ExternalInput")
    d_id = nc.dram_tensor("ident", [128, 128], f32, kind="ExternalInput")
    d_out = nc.dram_tensor("out", [S, DIM], f32, kind="ExternalOutput")

    with tile.TileContext(nc) as tc:
        with (
            tc.tile_pool(name="const", bufs=1) as cpool,
            tc.tile_pool(name="persist", bufs=1) as ppool,
        ):
            hsT = [cpool.tile([128, S], f32, tag=f"hsT{c}", name=f"hsT{c}") for c in range(4)]
            wq = [cpool.tile([128, DIM], f32, tag=f"wq{c}", name=f"wq{c}") for c in range(4)]
            wk = [cpool.tile([128, DIM], f32, tag=f"wk{c}", name=f"wk{c}") for c in range(4)]
            wv = [cpool.tile([128, DIM], f32, tag=f"wv{c}", name=f"wv{c}") for c in range(4)]
            wo = [cpool.tile([128, DIM], f32, tag=f"wo{c}", name=f"wo{c}") for c in range(4)]
            btt = cpool.tile([128, NBT], f32, tag="btt")
            ident = cpool.tile([128, 128], f32, tag="ident")
            for c in range(4):
                sl = slice(128 * c, 128 * (c + 1))
                nc.sync.dma_start(hsT[c][:], d_hsT[sl, :])
                nc.sync.dma_start(wq[c][:], d_wqT[sl, :])
                nc.sync.dma_start(wk[c][:], d_wkT[sl, :])
                nc.sync.dma_start(wv[c][:], d_wvT[sl, :])
                nc.sync.dma_start(wo[c][:], d_woT[sl, :])
            nc.sync.dma_start(btt[:], d_btt[:])
            nc.sync.dma_start(ident[:], d_id[:])

            # persistent activation layouts
            # QT2/KT2: partition (h%2)*64+d, free (h//2)*1024 + s
            QT2 = ppool.tile([128, 4 * S], f32, tag="QT2")
            KT2 = ppool.tile([128, 4 * S], f32, tag="KT2")
            V = [ppool.tile([128, DIM], f32, tag=f"V{st}", name=f"V{st}") for st in range(8)]
            # o_all tile j covers e in [128j, 128(j+1)), free = q
            o_all = [ppool.tile([128, S], f32, tag=f"oall{j}", name=f"oall{j}") for j in range(4)]

            # ---------------- phase 1: projections ----------------
            with tc.tile_pool(name="ps1", bufs=3, space="PSUM") as ps1:
                for dst, w in ((QT2, wq), (KT2, wk)):
                    for j in range(4):          # head pair
                        for st in range(2):     # s halves of 512
                            ps = ps1.tile([128, 512], f32, tag="proj")
                            for par in range(2):
                                h = 2 * j + par
                                for c in range(4):
                                    nc.tensor.matmul(
                                        ps[64 * par:64 * (par + 1), :],
                                        w[c][:, 64 * h:64 * (h + 1)],
                                        hsT[c][:, 512 * st:512 * (st + 1)],
                                        start=(c == 0), stop=(c == 3),
                                        tile_position=(0, 64 * par),
                                    )
                            nc.scalar.activation(
                                dst[:, j * S + 512 * st: j * S + 512 * (st + 1)],
                                ps[:], Act.Copy)
                for st in range(8):
                    ps = ps1.tile([128, 512], f32, tag="projv")
                    for c in range(4):
                        nc.tensor.matmul(
                            ps[:], hsT[c][:, 128 * st:128 * (st + 1)], wv[c][:],
                            start=(c == 0), stop=(c == 3))
                    nc.scalar.activation(V[st][:], ps[:], Act.Copy)

            # ---------------- phase 2: attention ----------------
            with (
                tc.tile_pool(name="sidxp", bufs=2) as sidxp,
                tc.tile_pool(name="work", bufs=2) as wk2,
                tc.tile_pool(name="pss", bufs=1, space="PSUM") as pss,
                tc.tile_pool(name="psqb", bufs=1, space="PSUM") as psqb,
                tc.tile_pool(name="pst", bufs=2, space="PSUM") as pst,
                tc.tile_pool(name="pso", bufs=2, space="PSUM") as pso,
            ):
                for qt in range(NQT):
                    sidx_t = sidxp.tile([128, 4 * NBT], i16, tag="sidx")
                    nc.sync.dma_start(
                        sidx_t[:], d_sidx[128 * qt:128 * (qt + 1), :])
                    for j in range(4):
                        ps_o = pso.tile([128, 128], f32, tag="pso")
                        for par in range(2):
                            h = 2 * j + par
                            base = 64 * par
                            bsl = slice(base, base + 64)
                            joff = j * S
                            lq = QT2[bsl, joff + 128 * qt: joff + 128 * (qt + 1)]

                            ps_s = pss.tile([128, S], f32, tag="scores")
                            for kb2 in range(2):
                                nc.tensor.matmul(
                                    ps_s[:, 512 * kb2:512 * (kb2 + 1)],
                                    lq,
                                    KT2[bsl, joff + 512 * kb2: joff + 512 * (kb2 + 1)],
                                    start=True, stop=True)
                            ps_qb = psqb.tile([128, 1024], f32, tag="qb")
                            nc.tensor.matmul(ps_qb[:, 0:512], lq,
                                             btt[bsl, 0:512],
                                             start=True, stop=True)
                            nc.tensor.matmul(ps_qb[:, 512:512 + 34], lq,
                                             btt[bsl, 512:NBT],
                                             start=True, stop=True)

                            # psum qb already holds qb_diamond - qb0
                            # (host-side subtraction); copy to SBUF for the
                            # gpsimd scatter, col 545 = qb0.
                            qbd = wk2.tile([128, NBT], f32, tag="qbd")
                            nc.scalar.activation(qbd[:, 0:512],
                                                 ps_qb[:, 0:512], Act.Copy)
                            nc.scalar.activation(qbd[:, 512:NBT],
                                                 ps_qb[:, 512:NBT], Act.Copy)
                            # scatter diamond into ctx (zero elsewhere);
                            # scatters are the ONLY Pool work (HW-measured
                            # ~9.7us each, they run concurrent with DVE topk)
                            ctx = wk2.tile([128, S], f32, tag="ctx")
                            qbd16 = qbd[:].bitcast(i16)
                            ctx16 = ctx[:].bitcast(i16)
                            for hf in range(2):
                                nc.gpsimd.local_scatter(
                                    ctx16[:, 1024 * hf:1024 * (hf + 1)],
                                    qbd16,
                                    sidx_t[:, 2 * NBT * hf:2 * NBT * (hf + 1)],
                                    channels=128, num_elems=1024,
                                    num_idxs=2 * NBT)

                            # qb0 background folded into the psum->SBUF copy
                            # as per-partition ACT bias (same fp32 rounding as
                            # (qk + qb0) + ctx); ctx added on Pool off the
                            # DVE critical path.
                            qb0 = qbd[:, 545:546]
                            sraw_p = wk2.tile([128, S], f32, tag="srawp")
                            nc.scalar.activation(sraw_p[:], ps_s[:],
                                                 Act.Identity, bias=qb0)
                            s_raw = wk2.tile([128, S], f32, tag="sraw")
                            nc.gpsimd.tensor_tensor(
                                s_raw[:], sraw_p[:], ctx[:], op=Alu.add)

                            # exact top-64 per row (last round needs no
                            # match_replace)
                            top64 = wk2.tile([128, 64], f32, tag="top64")
                            scratch = wk2.tile([128, S], f32, tag="scratch")
                            src = s_raw
                            for r in range(8):
                                nc.vector.max(
                                    out=top64[:, 8 * r:8 * (r + 1)],
                                    in_=src[:])
                                if r == 7:
                                    break
                                nc.vector.match_replace(
                                    out=scratch[:],
                                    in_to_replace=top64[:, 8 * r:8 * (r + 1)],
                                    in_values=src[:],
                                    imm_value=-1e30)
                                src = scratch

                            # softmax pieces
                            nsm = wk2.tile([128, 1], f32, tag="nsm")
                            nc.vector.tensor_scalar(
                                nsm[:], top64[:, 0:1], -SCALE, None,
                                op0=Alu.mult)
                            expP = wk2.tile([128, S], f32, tag="expP")
                            nc.scalar.activation(
                                expP[:], s_raw[:], Act.Exp,
                                bias=nsm[:], scale=SCALE)
                            # P = expP * (s_raw >= t64), sigma = row-sum(P)
                            P = wk2.tile([128, S], f32, tag="P")
                            sig = wk2.tile([128, 1], f32, tag="sig")
                            nc.vector.scalar_tensor_tensor(
                                P[:], s_raw[:], top64[:, 63:64], expP[:],
                                op0=Alu.is_ge, op1=Alu.mult,
                                accum_out=sig[:])
                            rs = wk2.tile([128, 1], f32, tag="rs")
                            nc.vector.reciprocal(rs[:], sig[:])
                            nc.gpsimd.tensor_scalar(
                                P[:], P[:], rs[:], None, op0=Alu.mult)

                            # transpose P in 128x128 blocks; PV accumulate
                            for kb in range(8):
                                ps_t = pst.tile([128, 128], f32, tag="pt")
                                nc.tensor.transpose(
                                    ps_t[:], P[:, 128 * kb:128 * (kb + 1)],
                                    ident[:])
                                pt_sb = wk2.tile([128, 128], f32, tag=f"pt{kb % 2}", name=f"ptsb{kb % 2}")
                                nc.scalar.activation(pt_sb[:], ps_t[:], Act.Copy)
                                nc.tensor.matmul(
                                    ps_o[base:base + 64, :],
                                    V[kb][:, 64 * h:64 * (h + 1)],
                                    pt_sb[:],
                                    start=(kb == 0), stop=(kb == 7),
                                    tile_position=(0, base))
                        nc.scalar.activation(
                            o_all[j][:, 128 * qt:128 * (qt + 1)],
                            ps_o[:], Act.Copy)

            # ---------------- phase 3: output projection ----------------
            with (
                tc.tile_pool(name="ps3", bufs=2, space="PSUM") as ps3,
                tc.tile_pool(name="outp", bufs=2) as outp,
            ):
                for st in range(8):
                    ps = ps3.tile([128, 512], f32, tag="out")
                    for c in range(4):
                        nc.tensor.matmul(
                            ps[:], o_all[c][:, 128 * st:128 * (st + 1)],
                            wo[c][:], start=(c == 0), stop=(c == 3))
                    ot = outp.tile([128, 512], f32, tag="ot")
                    nc.scalar.activation(ot[:], ps[:], Act.Copy)
                    nc.sync.dma_start(d_out[128 * st:128 * (st + 1), :], ot[:])

    nc.finalize()
    return nc


def kernel(hidden_states, Wqkv, Wo, bias_table, mask, qs0, qs1, ks0, ks1,
           topk, **_ignored):
    hidden_states = np.asarray(hidden_states, np.float32)
    Wqkv = np.asarray(Wqkv, np.float32)
    Wo = np.asarray(Wo, np.float32)
    bias_table = np.asarray(bias_table, np.float32)
    assert hidden_states.shape == (B, S, DIM), hidden_states.shape
    assert Wqkv.shape == (3 * H * D, DIM) and Wo.shape == (DIM, H * D)
    assert bias_table.shape == (NUM_BUCKETS ** 2, D)
    assert int(qs0) == GRID and int(qs1) == GRID
    assert int(ks0) == GRID and int(ks1) == GRID
    assert int(topk) == TOPK, topk

    hsT, wqT, wkT, wvT, woT, btt2, sidx, ident = _host_prep(
        hidden_states, Wqkv, Wo, bias_table)

    if "nc" not in _cache:
        _cache["nc"] = _build()
    nc = _cache["nc"]

    from concourse.bass_utils import run_bass_kernel_spmd
    shared = {"wqT": wqT, "wkT": wkT, "wvT": wvT, "woT": woT,
              "btt": btt2, "sidx": sidx, "ident": ident}
    in_maps = [dict(shared, hsT=np.ascontiguousarray(hsT[b]))
               for b in range(B)]
    res = run_bass_kernel_spmd(nc, in_maps, core_ids=list(range(B)))
    _cache["last_exec_time_ns"] = getattr(res, "exec_time_ns", None)
    out = np.stack([res.results[b]["out"] for b in range(B)], axis=0)
    return out
